# revision 17
# baseline (speedup 1.0000x reference)
"""AutoCorrelation (Autoformer-style) Trainium2 Bass kernel.

Sharding: data-parallel over batch — 8 batch elements -> 8 NeuronCores, no
collectives. Each core computes its [2048, 128] output slice independently.

Algorithm (validated against the reference in fp64 numpy to ~1e-7):
  * Single folded real-DFT matrix pair C,S = cos/sin(2*pi*i*j/2048) [1024x1024];
    symmetry of the matrix serves forward rfft and both inverse transforms.
  * Q/K/V are obtained by projecting the hidden spectrum (DFT linearity);
    biases enter only the DC bin (+T*b) — the Nyquist bin is carried separately.
  * acm = irfft(mean_h Qf conj(Kf)) via matmuls. Interior rfft weights (2/T)
    are applied uniformly; the resulting f=0 offset is constant per channel and
    cancels in both top-k ordering and softmax.
  * top-22 per channel: 3 rounds of VectorE max8 + match_replace; the 22nd
    delay from max_index; acm is kept in a permuted layout (lower half E-O,
    upper half E+O) and the delay index is remapped arithmetically.
  * The circular roll of V is a per-channel phase multiply in the frequency
    domain (shift theorem); softmax weight and 2/T fold into the phase tables.
  * float32r (full-rate fp32 matmul mode) on the accuracy-critical path,
    bf16 on the V/output path; elementwise complex products in fp32 split
    between VectorE and GpSimd.
"""
import os
import sys
import types
from contextlib import ExitStack

sys.path.insert(0, "/opt/trn_rl_repo")

import numpy as np

import concourse.bass as bass
import concourse.mybir as mybir
from concourse import bacc
from concourse.tile import TileContext
from concourse.bass_utils import run_bass_kernel_spmd

B, T, E, H = 8, 2048, 128, 4
NF = 1024
NCH = 8
AL = mybir.AluOpType
DT = mybir.dt
AF = mybir.ActivationFunctionType
AX = mybir.AxisListType

_CACHE = {}
LAST_EXEC_NS = None


def _wire_ntff_hook():
    if "antenv.axon_hooks" in sys.modules:
        return
    try:
        mod = types.ModuleType("antenv.axon_hooks")
        _h = [None]
        mod.set_axon_ntff_profile_hook = lambda h: _h.__setitem__(0, h)
        mod.get_axon_ntff_profile_hook = lambda: _h[0]
        sys.modules["antenv.axon_hooks"] = mod
        import antenv
        antenv.axon_hooks = mod
        from trn_agent_boot.trn_boot import _ntff_profile_via_ctypes
        mod.set_axon_ntff_profile_hook(_ntff_profile_via_ctypes("/opt/axon/libaxon_pjrt.so"))
    except Exception:
        pass


def _host_consts():
    i = np.arange(NF, dtype=np.float64)
    ang = np.outer(i, i) * (2.0 * np.pi / T)
    return {
        "cs": np.cos(ang).astype(np.float32),
        "sn": np.sin(ang).astype(np.float32),
        "altf": ((-1.0) ** np.arange(NF)).astype(np.float32)[None, :],
        "altp": ((-1.0) ** np.arange(128)).astype(np.float32)[:, None],
        "one": np.ones((1, 1), np.float32),
        "mhalf": np.full((1, 1), -0.5, np.float32),
        "mhrow": np.full((1, NF), -0.5, np.float32),
    }


def _build():
    nc = bacc.Bacc("TRN2", target_bir_lowering=False, debug=False, num_devices=1)
    f32, f32r, bf16, i32, u32 = DT.float32, DT.float32r, DT.bfloat16, DT.int32, DT.uint32

    x_d = nc.dram_tensor("x", [T, E], f32r, kind="ExternalInput")
    xr_d = nc.dram_tensor("xr", [NF, E], f32r, kind="ExternalInput")
    cs_d = nc.dram_tensor("cs", [NF, NF], f32r, kind="ExternalInput")
    sn_d = nc.dram_tensor("sn", [NF, NF], f32r, kind="ExternalInput")
    altf_d = nc.dram_tensor("altf", [1, NF], f32r, kind="ExternalInput")
    altp_d = nc.dram_tensor("altp", [128, 1], f32r, kind="ExternalInput")
    one_d = nc.dram_tensor("one", [1, 1], f32r, kind="ExternalInput")
    mhalf_d = nc.dram_tensor("mhalf", [1, 1], f32r, kind="ExternalInput")
    mhrow_d = nc.dram_tensor("mhrow", [1, NF], f32r, kind="ExternalInput")
    wqk_d = nc.dram_tensor("wqk", [H, E, 256], f32r, kind="ExternalInput")
    wv_d = nc.dram_tensor("wv", [H, E, E], f32r, kind="ExternalInput")
    wo_d = nc.dram_tensor("wo", [H, E, E], f32, kind="ExternalInput")
    bqk_d = nc.dram_tensor("bqk", [1, 2 * H * E], f32, kind="ExternalInput")  # T*bq | T*bk (h-major)
    bv_d = nc.dram_tensor("bv", [E, H], f32, kind="ExternalInput")        # T*bv
    bo_d = nc.dram_tensor("bo", [E, 1], f32, kind="ExternalInput")
    lo_d = nc.dram_tensor("out_lo", [E, NF], f32, kind="ExternalOutput")
    hi_d = nc.dram_tensor("out_hi", [E, NF], f32, kind="ExternalOutput")
    o1024_d = nc.dram_tensor("out_1024", [E, 1], f32, kind="ExternalOutput")

    c32 = lambda ap: ap.bitcast(DT.float32)
    with TileContext(nc) as tc, ExitStack() as ctx:
        pool = ctx.enter_context(tc.tile_pool(name="main", bufs=1))
        pool2 = ctx.enter_context(tc.tile_pool(name="rot", bufs=2))
        ppool = ctx.enter_context(tc.tile_pool(name="ps", bufs=6, space="PSUM"))
        prow = ctx.enter_context(tc.tile_pool(name="psrow", bufs=2, space="PSUM"))

        # ---------------- loads ----------------
        cs_sb = pool.tile([128, NCH * NF], f32r, tag="cs")
        sn_sb = pool.tile([128, NCH * NF], f32r, tag="sn")
        for i in range(NCH):
            nc.sync.dma_start(cs_sb[:, i * NF:(i + 1) * NF], cs_d[128 * i:128 * (i + 1), :])
            nc.sync.dma_start(sn_sb[:, i * NF:(i + 1) * NF], sn_d[128 * i:128 * (i + 1), :])

        xt = pool.tile([128, NCH * E], f32r, tag="scr_a")
        xrt = pool.tile([128, NCH * E], f32r, tag="scr_b")
        for a in range(NCH):
            nc.sync.dma_start(xt[:, a * E:(a + 1) * E], x_d[128 * a:128 * (a + 1), :])
            nc.sync.dma_start(xrt[:, a * E:(a + 1) * E], xr_d[128 * a:128 * (a + 1), :])
        xnyq = pool.tile([1, E], f32r, tag="xnyq")
        nc.sync.dma_start(xnyq[:], x_d[NF:NF + 1, :])

        altf_sb = pool.tile([1, NF], f32r, tag="altf")
        nc.sync.dma_start(altf_sb[:], altf_d[:])
        altp_sb = pool.tile([128, 1], f32r, tag="altp")
        nc.sync.dma_start(altp_sb[:], altp_d[:])
        one_sb = pool.tile([1, 1], f32r, tag="one")
        nc.sync.dma_start(one_sb[:], one_d[:])
        mhalf_sb = pool.tile([1, 1], f32r, tag="mhalf")
        nc.sync.dma_start(mhalf_sb[:], mhalf_d[:])
        mhrow_sb = pool.tile([1, NF], f32r, tag="mhrow")
        nc.sync.dma_start(mhrow_sb[:], mhrow_d[:])

        wqk_sb = pool.tile([128, H * 256], f32r, tag="wqk")
        for h in range(H):
            nc.sync.dma_start(wqk_sb[:, h * 256:(h + 1) * 256], wqk_d[h])
        wv_sb = pool.tile([128, H * E], f32r, tag="wv")
        for h in range(H):
            nc.sync.dma_start(wv_sb[:, h * E:(h + 1) * E], wv_d[h])
        wo32 = pool2.tile([128, H * E], f32, tag="vre")
        for h in range(H):
            nc.sync.dma_start(wo32[:, h * E:(h + 1) * E], wo_d[h])
        wo_sb = pool.tile([128, H * E], bf16, tag="wo")
        nc.scalar.copy(wo_sb[:], wo32[:])

        bqk_sb = pool.tile([1, 2 * H * E], f32, tag="bqk")
        nc.sync.dma_start(bqk_sb[:], bqk_d[:])
        bv_sb = pool.tile([128, H], f32, tag="bv")
        nc.sync.dma_start(bv_sb[:], bv_d[:])
        bo_sb = pool.tile([128, 1], f32, tag="bo")
        nc.sync.dma_start(bo_sb[:], bo_d[:])

        # ---------------- fold ----------------
        xc = pool.tile([128, NCH * E], f32r, tag="xc")
        xs = pool.tile([128, NCH * E], f32r, tag="xs")
        nc.vector.tensor_add(xc[:], xt[:], xrt[:])
        nc.vector.tensor_sub(xs[:], xrt[:], xt[:])
        nc.vector.tensor_scalar_mul(xc[0:1, 0:E], xc[0:1, 0:E], 0.5)

        # ---------------- forward DFT ----------------
        hre = pool.tile([128, NF], f32r, tag="hre")
        him = pool.tile([128, NF], f32r, tag="him")
        hn = pool.tile([128, 1], f32r, tag="hn")
        for s in range(2):
            sl = slice(s * 512, (s + 1) * 512)
            hre_ps = ppool.tile([128, 512], f32, tag="bank")
            him_ps = ppool.tile([128, 512], f32, tag="bank")
            for a in range(NCH):
                nc.tensor.matmul(hre_ps[:], xc[:, a * E:(a + 1) * E],
                                 cs_sb[:, a * NF + s * 512: a * NF + (s + 1) * 512],
                                 start=(a == 0), stop=False)
            nc.tensor.matmul(hre_ps[:], xnyq[:], altf_sb[:, sl], start=False, stop=True)
            for a in range(NCH):
                nc.tensor.matmul(him_ps[:], xs[:, a * E:(a + 1) * E],
                                 sn_sb[:, a * NF + s * 512: a * NF + (s + 1) * 512],
                                 start=(a == 0), stop=(a == NCH - 1))
            nc.vector.tensor_copy(hre[:, sl], hre_ps[:])
            nc.vector.tensor_copy(him[:, sl], him_ps[:])
        hn_ps = prow.tile([128, 1], f32, tag="row")
        for a in range(NCH):
            nc.tensor.matmul(hn_ps[:], c32(xc[:, a * E:(a + 1) * E]), c32(altp_sb[:]),
                             start=(a == 0), stop=False)
        nc.tensor.matmul(hn_ps[:], c32(xnyq[:]), c32(one_sb[:]), start=False, stop=True)
        nc.vector.tensor_copy(hn[:], hn_ps[:])

        # ---------------- QK projections (transposed out) + products ----------------
        pre_r = pool.tile([128, NCH * E], f32r, tag="prer")
        pim_r = pool.tile([128, NCH * E], f32r, tag="pimr")
        pre_t = pool.tile([128, NCH * E], f32, tag="scr_a")
        pim_t = pool.tile([128, NCH * E], f32, tag="scr_b")
        qn_row = pool.tile([1, E], f32, tag="qnrow")     # per-h rows accumulated into pn
        kn_row = pool.tile([1, E], f32, tag="knrow")
        pn_row = pool.tile([1, E], f32, tag="pnrow")
        pn_row_r = pool.tile([1, E], f32r, tag="pnrowr")
        pnw = pool.tile([1, E], f32, tag="pnw")

        for h in range(H):
            qkt_c = []
            for c in range(2):
                qkt = pool2.tile([128, NCH * 256], f32, tag="qkt")  # (j, qk, e')
                qkt_c.append(qkt)
                hsrc = hre if c == 0 else him
                for g in range(4):  # groups of 2 chunks -> one [128,512] psum
                    qk_ps = ppool.tile([128, 512], f32, tag="bank")
                    for jj in range(2):
                        j = g * 2 + jj
                        nc.tensor.matmul(qk_ps[:, jj * 256:(jj + 1) * 256],
                                         hsrc[:, j * 128:(j + 1) * 128],
                                         wqk_sb[:, h * 256:(h + 1) * 256],
                                         start=True, stop=True)
                    nc.scalar.copy(qkt[:, g * 512:(g + 1) * 512], qk_ps[:])
            # DC bias: f=0 is chunk0,row0 -> partition 0, cols (j=0, q|k, :)
            nc.vector.tensor_add(qkt_c[0][0:1, 0:128], qkt_c[0][0:1, 0:128], bqk_sb[0:1, h * E:(h + 1) * E])
            nc.vector.tensor_add(qkt_c[0][0:1, 128:256], qkt_c[0][0:1, 128:256], bqk_sb[0:1, H * E + h * E:H * E + (h + 1) * E])

            view_re = qkt_c[0][:].rearrange("p (j k e) -> k p j e", j=NCH, k=2)
            view_im = qkt_c[1][:].rearrange("p (j k e) -> k p j e", j=NCH, k=2)
            qr, kr = view_re[0], view_re[1]
            qi, ki = view_im[0], view_im[1]
            r3 = lambda ap: ap.rearrange("p (j e) -> p j e", j=NCH)

            eng_a = nc.vector if h % 2 == 0 else nc.gpsimd
            eng_b = nc.gpsimd if h % 2 == 0 else nc.vector
            t1 = pool2.tile([128, NCH * E], f32, tag="ptmp")
            eng_a.tensor_tensor(r3(t1[:]), qr, kr, AL.mult)
            t2 = pool2.tile([128, NCH * E], f32, tag="ptmp")
            eng_a.tensor_tensor(r3(t2[:]), qi, ki, AL.mult)
            t3 = pool2.tile([128, NCH * E], f32, tag="ptmq")
            eng_b.tensor_tensor(r3(t3[:]), qi, kr, AL.mult)
            t4 = pool2.tile([128, NCH * E], f32, tag="ptmq")
            eng_b.tensor_tensor(r3(t4[:]), qr, ki, AL.mult)
            if h == 0:
                eng_a.tensor_add(pre_t[:], t1[:], t2[:])
                eng_b.tensor_sub(pim_t[:], t3[:], t4[:])
            else:
                eng_a.tensor_add(pre_t[:], pre_t[:], t1[:])
                eng_a.tensor_add(pre_t[:], pre_t[:], t2[:])
                eng_b.tensor_add(pim_t[:], pim_t[:], t3[:])
                eng_b.tensor_sub(pim_t[:], pim_t[:], t4[:])
            if h == H - 1:
                s_scale = 2.0 / (T * H)
                nc.vector.tensor_scalar_mul(pre_r[:], pre_t[:], s_scale)
                nc.gpsimd.tensor_scalar_mul(pim_r[:], pim_t[:], s_scale)

            # Nyquist rows: qn_row = hn^T @ wq_h ; kn_row = hn^T @ wk_h
            r_ps = prow.tile([1, 256], f32, tag="row")
            nc.tensor.matmul(r_ps[:], c32(hn[:]), c32(wqk_sb[:, h * 256:(h + 1) * 256]), start=True, stop=True)
            nc.scalar.copy(qn_row[:], r_ps[:, 0:128])
            nc.scalar.copy(kn_row[:], r_ps[:, 128:256])
            if h == 0:
                nc.vector.scalar_tensor_tensor(pn_row[:], qn_row[:], 1.0 / (T * H), kn_row[:], AL.mult, AL.mult)
            else:
                nc.vector.scalar_tensor_tensor(pnw[:], qn_row[:], 1.0 / (T * H), kn_row[:], AL.mult, AL.mult)
                dst = pn_row_r if h == H - 1 else pn_row
                nc.vector.tensor_add(dst[:], pn_row[:], pnw[:])

        # ---------------- acm inverse ----------------
        eo_sb = pool2.tile([128, T], f32, tag="qkt")
        for s in range(2):
            sl = slice(s * 512, (s + 1) * 512)
            e_ps = ppool.tile([128, 512], f32, tag="bank")
            o_ps = ppool.tile([128, 512], f32, tag="bank")
            for j in range(NCH):
                nc.tensor.matmul(e_ps[:], pre_r[:, j * E:(j + 1) * E],
                                 cs_sb[:, j * NF + s * 512: j * NF + (s + 1) * 512],
                                 start=(j == 0), stop=False)
            nc.tensor.matmul(e_ps[:], pn_row_r[:], altf_sb[:, sl], start=False, stop=True)
            for j in range(NCH):
                nc.tensor.matmul(o_ps[:], pim_r[:, j * E:(j + 1) * E],
                                 sn_sb[:, j * NF + s * 512: j * NF + (s + 1) * 512],
                                 start=(j == 0), stop=(j == NCH - 1))
            e_sb = pool2.tile([128, 512], f32, tag="ecp")
            nc.scalar.copy(e_sb[:], e_ps[:])
            nc.vector.tensor_sub(eo_sb[:, sl], e_sb[:], o_ps[:])
            if s == 0:
                nc.vector.tensor_add(eo_sb[:, NF + 1:NF + 512], e_sb[:, 1:], o_ps[:, 1:])
            else:
                nc.vector.tensor_add(eo_sb[:, NF + 512:2 * NF], e_sb[:], o_ps[:])
        a1024_ps = prow.tile([128, 1], f32, tag="row")
        for j in range(NCH):
            nc.tensor.matmul(a1024_ps[:], c32(pre_r[:, j * E:(j + 1) * E]), c32(altp_sb[:]),
                             start=(j == 0), stop=False)
        nc.tensor.matmul(a1024_ps[:], c32(pn_row_r[:]), c32(one_sb[:]), start=False, stop=True)
        nc.scalar.copy(eo_sb[:, NF:NF + 1], a1024_ps[:])

        # ---------------- top-k ----------------
        vals = pool.tile([128, 24], f32, tag="vals")
        nc.vector.max(vals[:, 0:8], eo_sb[:])
        nc.vector.match_replace(eo_sb[:], vals[:, 0:8], eo_sb[:], -1e30)
        nc.vector.max(vals[:, 8:16], eo_sb[:])
        nc.vector.match_replace(eo_sb[:], vals[:, 8:16], eo_sb[:], -1e30)
        nc.vector.max(vals[:, 16:24], eo_sb[:])
        idx8 = pool.tile([128, 8], u32, tag="idx8")
        nc.vector.max_index(idx8[:], vals[:, 16:24], eo_sb[:])

        c_i = pool.tile([128, 1], i32, tag="ci")
        nc.vector.tensor_copy(c_i[:], idx8[:, 5:6].bitcast(i32))
        c_neg = pool.tile([128, 1], i32, tag="cneg")
        nc.vector.tensor_scalar(c_neg[:], c_i[:], -1, 3072, AL.mult, AL.add)
        mask = pool.tile([128, 1], i32, tag="mask")
        nc.vector.tensor_scalar(mask[:], c_i[:], 1024, None, AL.is_gt)
        d_i = pool.tile([128, 1], i32, tag="di")
        nc.vector.select(d_i[:], mask[:], c_neg[:], c_i[:])
        d_f = pool.tile([128, 1], f32, tag="df")
        nc.vector.tensor_copy(d_f[:], d_i[:])

        negv0 = pool.tile([128, 1], f32, tag="negv0")
        nc.vector.tensor_scalar_mul(negv0[:], vals[:, 0:1], -1.0)
        expv = pool.tile([128, 24], f32, tag="expv")
        nc.scalar.activation(expv[:, 0:22], vals[:, 0:22], AF.Exp, bias=negv0[:])
        den = pool.tile([128, 1], f32, tag="den")
        nc.vector.tensor_reduce(den[:], expv[:, 0:22], AX.X, AL.add)
        rden = pool.tile([128, 1], f32, tag="rden")
        nc.vector.reciprocal(rden[:], den[:])
        wgt = pool.tile([128, 1], f32, tag="wgt")
        nc.vector.tensor_mul(wgt[:], expv[:, 21:22], rden[:])

        # ---------------- phases ----------------
        negpi = pool.tile([128, 1], f32, tag="negpi")
        nc.gpsimd.memset(negpi[:], float(-np.pi))
        io_i = pool.tile([128, NF], i32, tag="scr_a")
        nc.gpsimd.iota(io_i[:], pattern=[[1, NF]], base=0, channel_multiplier=0)
        io_f = pool.tile([128, NF], f32, tag="scr_b")
        nc.vector.tensor_copy(io_f[:], io_i[:])
        cw = pool.tile([128, NF], bf16, tag="cw")
        sw = pool.tile([128, NF], bf16, tag="sw")
        # -Sin(pi/1024 * ((f*d + off)&2047) - pi) = sin/cos(2*pi*f*d/2048); the
        # leading minus is folded into the -2/T weight scale below.
        for off, dst in ((0.0, sw), (512.0, cw)):
            mf = pool2.tile([128, NF], f32, tag="ptmp")
            if off == 0.0:
                nc.vector.tensor_scalar(mf[:], io_f[:], d_f[:], None, AL.mult)
            else:
                nc.vector.tensor_scalar(mf[:], io_f[:], d_f[:], off, AL.mult, AL.add)
            mi = pool2.tile([128, NF], i32, tag="ptmp")
            nc.vector.tensor_copy(mi[:], mf[:])
            nc.vector.tensor_scalar(mi[:], mi[:], 2047, None, AL.bitwise_and)
            mf2 = pool2.tile([128, NF], f32, tag="ptmp")
            nc.vector.tensor_copy(mf2[:], mi[:])
            ph = pool2.tile([128, NF], f32, tag="ptmp")
            nc.scalar.activation(ph[:], mf2[:], AF.Sin,
                                 scale=float(np.pi / 1024.0), bias=negpi[:])
            nc.vector.tensor_scalar(dst[:], ph[:], wgt[:], -2.0 / T, AL.mult, AL.mult)
        # nyquist scale: (1-2*(d&1)) * wgt / T
        par_i = pool.tile([128, 1], i32, tag="par")
        nc.vector.tensor_scalar(par_i[:], d_i[:], 1, None, AL.bitwise_and)
        parf = pool.tile([128, 1], f32, tag="parf")
        nc.vector.tensor_copy(parf[:], par_i[:])
        nc.vector.tensor_scalar(parf[:], parf[:], -2.0, 1.0, AL.mult, AL.add)
        nys = pool.tile([128, 1], f32, tag="nys")
        nc.vector.tensor_scalar(nys[:], parf[:], wgt[:], 1.0 / T, AL.mult, AL.mult)

        # ---------------- V spectra + phase multiply (all h) ----------------
        gre_t = {}
        gim_t = {}
        gn_cols = pool.tile([128, H], bf16, tag="gncols")
        for h in range(H):
            vre = pool2.tile([128, NF], bf16, tag="vre")
            vim = pool2.tile([128, NF], bf16, tag="vim")
            for c in range(2):
                hsrc = hre if c == 0 else him
                dst = vre if c == 0 else vim
                for s in range(2):
                    v_ps = ppool.tile([128, 512], f32, tag="bank")
                    nc.tensor.matmul(v_ps[:], wv_sb[:, h * E:(h + 1) * E],
                                     hsrc[:, s * 512:(s + 1) * 512], start=True, stop=True)
                    if c == 0 and s == 0:
                        nc.vector.tensor_add(v_ps[:, 0:1], v_ps[:, 0:1], bv_sb[:, h:h + 1])
                    nc.scalar.copy(dst[:, s * 512:(s + 1) * 512], v_ps[:])
            vn_ps = prow.tile([128, 1], f32, tag="row")
            nc.tensor.matmul(vn_ps[:], c32(wv_sb[:, h * E:(h + 1) * E]), c32(hn[:]), start=True, stop=True)
            vn_col = pool.tile([128, 1], f32, tag=f"vn{h}")
            nc.scalar.copy(vn_col[:], vn_ps[:])
            nc.vector.tensor_mul(gn_cols[:, h:h + 1], vn_col[:], nys[:])

            gre = pool.tile([128, NF], bf16, tag=f"gre{h}")
            gim = pool.tile([128, NF], bf16, tag=f"gim{h}")
            gre_t[h], gim_t[h] = gre, gim
            m1 = pool2.tile([128, NF], bf16, tag="gm1")
            m2 = pool2.tile([128, NF], bf16, tag="gm2")
            nc.vector.tensor_mul(m1[:], vre[:], cw[:])
            nc.vector.tensor_mul(m2[:], vim[:], sw[:])
            nc.vector.tensor_sub(gre[:], m1[:], m2[:])
            m3 = pool2.tile([128, NF], bf16, tag="gm1")
            m4 = pool2.tile([128, NF], bf16, tag="gm2")
            nc.gpsimd.tensor_mul(m3[:], vre[:], sw[:])
            nc.gpsimd.tensor_mul(m4[:], vim[:], cw[:])
            nc.gpsimd.tensor_add(gim[:], m3[:], m4[:])

        # ---------------- output projection ----------------
        of_re = pool.tile([128, NCH * E], f32r, tag="ofre")
        of_im = pool.tile([128, NCH * E], f32r, tag="ofim")
        for j in range(NCH):
            for c in range(2):
                of_ps = ppool.tile([128, 512], f32, tag="bank")
                gt = gre_t if c == 0 else gim_t
                for h in range(H):
                    nc.tensor.matmul(of_ps[:, 0:128], gt[h][:, j * 128:(j + 1) * 128],
                                     wo_sb[:, h * E:(h + 1) * E],
                                     start=(h == 0), stop=(h == H - 1))
                dst = of_re if c == 0 else of_im
                nc.vector.tensor_copy(dst[:, j * E:(j + 1) * E], of_ps[:, 0:128])
        ofn_ps = prow.tile([1, E], f32, tag="row")
        for h in range(H):
            nc.tensor.matmul(ofn_ps[:], gn_cols[:, h:h + 1], wo_sb[:, h * E:(h + 1) * E],
                             start=(h == 0), stop=(h == H - 1))
        ofn_row = pool.tile([1, E], f32r, tag="ofnrow")
        nc.vector.tensor_copy(ofn_row[:], ofn_ps[:])

        # ---------------- final inverse ----------------
        for s in range(2):
            sl = slice(s * 512, (s + 1) * 512)
            ep_ps = ppool.tile([128, 512], f32, tag="bank")
            op_ps = ppool.tile([128, 512], f32, tag="bank")
            for j in range(NCH):
                nc.tensor.matmul(ep_ps[:], of_re[:, j * E:(j + 1) * E],
                                 cs_sb[:, j * NF + s * 512: j * NF + (s + 1) * 512],
                                 start=(j == 0), stop=False)
            nc.tensor.matmul(ep_ps[:], ofn_row[:], altf_sb[:, sl], start=False, stop=False)
            # -0.5 * Of_re[0, f''] constant-in-t correction (f=0 weight fix)
            nc.tensor.matmul(ep_ps[:], of_re[0:1, 0:E], mhrow_sb[:, sl], start=False, stop=True)
            for j in range(NCH):
                nc.tensor.matmul(op_ps[:], of_im[:, j * E:(j + 1) * E],
                                 sn_sb[:, j * NF + s * 512: j * NF + (s + 1) * 512],
                                 start=(j == 0), stop=(j == NCH - 1))
            ep_sb = pool2.tile([128, 512], f32, tag="ecp")
            nc.scalar.copy(ep_sb[:], ep_ps[:])
            out_lo = pool2.tile([128, 512], f32, tag="outlo")
            out_hi = pool2.tile([128, 512], f32, tag="outhi")
            nc.vector.scalar_tensor_tensor(out_lo[:], ep_sb[:], bo_sb[:], op_ps[:], AL.add, AL.subtract)
            nc.vector.scalar_tensor_tensor(out_hi[:], ep_sb[:], bo_sb[:], op_ps[:], AL.add, AL.add)
            nc.sync.dma_start(lo_d[:, sl], out_lo[:])
            nc.sync.dma_start(hi_d[:, sl], out_hi[:])
        # t = 1024 row
        o1_ps = prow.tile([128, 1], f32, tag="row")
        for j in range(NCH):
            nc.tensor.matmul(o1_ps[:], c32(of_re[:, j * E:(j + 1) * E]), c32(altp_sb[:]),
                             start=(j == 0), stop=False)
        nc.tensor.matmul(o1_ps[:], c32(ofn_row[:]), c32(one_sb[:]), start=False, stop=False)
        nc.tensor.matmul(o1_ps[:], c32(of_re[0:1, 0:E]), c32(mhalf_sb[:]), start=False, stop=True)
        o1_sb = pool.tile([128, 1], f32, tag="o1sb")
        nc.vector.tensor_scalar(o1_sb[:], o1_ps[:], bo_sb[:], None, AL.add)
        nc.sync.dma_start(o1024_d[:], o1_sb[:])

    nc.compile()
    return nc


def _get_nc():
    if "nc" not in _CACHE:
        _wire_ntff_hook()
        _CACHE["nc"] = _build()
    return _CACHE["nc"]


def kernel(hidden_states, wq, bq, wk, bk, wv, bv, wo, bo):
    global LAST_EXEC_NS
    nc = _get_nc()
    consts = _CACHE.setdefault("consts", _host_consts())

    hs = np.ascontiguousarray(hidden_states, dtype=np.float32)
    wqk = np.concatenate([wq.transpose(2, 0, 1), wk.transpose(2, 0, 1)], axis=2)  # [H, E, 256]
    wv_h = np.ascontiguousarray(wv.transpose(2, 0, 1), dtype=np.float32)          # [H, E, E]
    wo_h = np.ascontiguousarray(wo.transpose(1, 0, 2), dtype=np.float32)          # [H, E, E]
    bqk = np.concatenate([(T * bq.T).reshape(-1), (T * bk.T).reshape(-1)])[None, :].astype(np.float32)  # [1, 2*H*E]
    bv_s = np.ascontiguousarray(T * bv, dtype=np.float32)                         # [E, H]
    bo_c = np.ascontiguousarray(bo, dtype=np.float32)[:, None]                    # [E, 1]

    in_maps = []
    for b in range(B):
        x = hs[b]
        xr = np.concatenate([x[0:1], x[:0:-1]])[:NF]
        in_maps.append({
            "x": x, "xr": np.ascontiguousarray(xr),
            "cs": consts["cs"], "sn": consts["sn"], "altf": consts["altf"],
            "altp": consts["altp"], "one": consts["one"], "mhalf": consts["mhalf"],
            "mhrow": consts["mhrow"],
            "wqk": np.ascontiguousarray(wqk, dtype=np.float32), "wv": wv_h,
            "wo": wo_h, "bqk": bqk, "bv": bv_s, "bo": bo_c,
        })

    trace = bool(int(os.environ.get("BASS_KERNEL_TRACE", "0")))
    res = run_bass_kernel_spmd(nc, in_maps, core_ids=list(range(B)), trace=trace)
    LAST_EXEC_NS = res.exec_time_ns
    _CACHE["last_res"] = res

    out = np.empty((B, T, E), dtype=np.float32)
    for b in range(B):
        r = res.results[b]
        out[b, 0:NF] = r["out_lo"].T
        out[b, NF] = r["out_1024"][:, 0]
        out[b, NF + 1:] = r["out_hi"][:, 1:NF][:, ::-1].T
    return out


# revision 20
# speedup vs baseline: 1.0012x; 1.0012x over previous
"""AutoCorrelation (Autoformer-style) Trainium2 Bass kernel.

Sharding: data-parallel over batch — 8 batch elements -> 8 NeuronCores, no
collectives. Each core computes its [2048, 128] output slice independently.

Algorithm (validated against the reference in fp64 numpy to ~1e-7):
  * Single folded real-DFT matrix pair C,S = cos/sin(2*pi*i*j/2048) [1024x1024];
    symmetry of the matrix serves forward rfft and both inverse transforms.
  * Q/K/V are obtained by projecting the hidden spectrum (DFT linearity);
    biases enter only the DC bin (+T*b) — the Nyquist bin is carried separately.
  * acm = irfft(mean_h Qf conj(Kf)) via matmuls. Interior rfft weights (2/T)
    are applied uniformly; the resulting f=0 offset is constant per channel and
    cancels in both top-k ordering and softmax.
  * top-22 per channel: 3 rounds of VectorE max8 + match_replace; the 22nd
    delay from max_index; acm is kept in a permuted layout (lower half E-O,
    upper half E+O) and the delay index is remapped arithmetically.
  * The circular roll of V is a per-channel phase multiply in the frequency
    domain (shift theorem); softmax weight and 2/T fold into the phase tables.
  * float32r (full-rate fp32 matmul mode) on the accuracy-critical path,
    bf16 on the V/output path; elementwise complex products in fp32 split
    between VectorE and GpSimd.
"""
import os
import sys
import types
from contextlib import ExitStack

sys.path.insert(0, "/opt/trn_rl_repo")

import numpy as np

import concourse.bass as bass
import concourse.mybir as mybir
from concourse import bacc
from concourse.tile import TileContext
from concourse.bass_utils import run_bass_kernel_spmd

B, T, E, H = 8, 2048, 128, 4
NF = 1024
NCH = 8
AL = mybir.AluOpType
DT = mybir.dt
AF = mybir.ActivationFunctionType
AX = mybir.AxisListType

_CACHE = {}
LAST_EXEC_NS = None


def _wire_ntff_hook():
    if "antenv.axon_hooks" in sys.modules:
        return
    try:
        mod = types.ModuleType("antenv.axon_hooks")
        _h = [None]
        mod.set_axon_ntff_profile_hook = lambda h: _h.__setitem__(0, h)
        mod.get_axon_ntff_profile_hook = lambda: _h[0]
        sys.modules["antenv.axon_hooks"] = mod
        import antenv
        antenv.axon_hooks = mod
        from trn_agent_boot.trn_boot import _ntff_profile_via_ctypes
        mod.set_axon_ntff_profile_hook(_ntff_profile_via_ctypes("/opt/axon/libaxon_pjrt.so"))
    except Exception:
        pass


def _host_consts():
    i = np.arange(NF, dtype=np.float64)
    ang = np.outer(i, i) * (2.0 * np.pi / T)
    return {
        "cs": np.cos(ang).astype(np.float32),
        "sn": np.sin(ang).astype(np.float32),
        "altf": ((-1.0) ** np.arange(NF)).astype(np.float32)[None, :],
        "altp": ((-1.0) ** np.arange(128)).astype(np.float32)[:, None],
        "one": np.ones((1, 1), np.float32),
        "mhalf": np.full((1, 1), -0.5, np.float32),
        "mhrow": np.full((1, NF), -0.5, np.float32),
    }


def _build():
    nc = bacc.Bacc("TRN2", target_bir_lowering=False, debug=False, num_devices=1)
    f32, f32r, bf16, i32, u32 = DT.float32, DT.float32r, DT.bfloat16, DT.int32, DT.uint32

    x_d = nc.dram_tensor("x", [T, E], f32r, kind="ExternalInput")
    xr_d = nc.dram_tensor("xr", [NF, E], f32r, kind="ExternalInput")
    cs_d = nc.dram_tensor("cs", [NF, NF], f32r, kind="ExternalInput")
    sn_d = nc.dram_tensor("sn", [NF, NF], f32r, kind="ExternalInput")
    altf_d = nc.dram_tensor("altf", [1, NF], f32r, kind="ExternalInput")
    altp_d = nc.dram_tensor("altp", [128, 1], f32r, kind="ExternalInput")
    one_d = nc.dram_tensor("one", [1, 1], f32r, kind="ExternalInput")
    mhalf_d = nc.dram_tensor("mhalf", [1, 1], f32r, kind="ExternalInput")
    mhrow_d = nc.dram_tensor("mhrow", [1, NF], f32r, kind="ExternalInput")
    wqk_d = nc.dram_tensor("wqk", [H, E, 256], f32r, kind="ExternalInput")
    wv_d = nc.dram_tensor("wv", [H, E, E], f32r, kind="ExternalInput")
    wo_d = nc.dram_tensor("wo", [H, E, E], f32, kind="ExternalInput")
    bqk_d = nc.dram_tensor("bqk", [1, 2 * H * E], f32, kind="ExternalInput")  # T*bq | T*bk (h-major)
    bv_d = nc.dram_tensor("bv", [E, H], f32, kind="ExternalInput")        # T*bv
    bo_d = nc.dram_tensor("bo", [E, 1], f32, kind="ExternalInput")
    lo_d = nc.dram_tensor("out_lo", [E, NF], f32, kind="ExternalOutput")
    hi_d = nc.dram_tensor("out_hi", [E, NF], f32, kind="ExternalOutput")
    o1024_d = nc.dram_tensor("out_1024", [E, 1], f32, kind="ExternalOutput")

    c32 = lambda ap: ap.bitcast(DT.float32)
    with TileContext(nc) as tc, ExitStack() as ctx:
        pool = ctx.enter_context(tc.tile_pool(name="main", bufs=1))
        pool2 = ctx.enter_context(tc.tile_pool(name="rot", bufs=2))
        ppool = ctx.enter_context(tc.tile_pool(name="ps", bufs=6, space="PSUM"))
        prow = ctx.enter_context(tc.tile_pool(name="psrow", bufs=2, space="PSUM"))

        # ---------------- loads ----------------
        cs_sb = pool.tile([128, NCH * NF], f32r, tag="cs")
        sn_sb = pool.tile([128, NCH * NF], f32r, tag="sn")

        xt = pool.tile([128, NCH * E], f32r, tag="scr_a")
        xrt = pool.tile([128, NCH * E], f32r, tag="scr_b")
        for a in range(NCH):
            nc.sync.dma_start(xt[:, a * E:(a + 1) * E], x_d[128 * a:128 * (a + 1), :])
            nc.sync.dma_start(xrt[:, a * E:(a + 1) * E], xr_d[128 * a:128 * (a + 1), :])
        xnyq = pool.tile([1, E], f32r, tag="xnyq")
        nc.sync.dma_start(xnyq[:], x_d[NF:NF + 1, :])

        altf_sb = pool.tile([1, NF], f32r, tag="altf")
        nc.sync.dma_start(altf_sb[:], altf_d[:])
        altp_sb = pool.tile([128, 1], f32r, tag="altp")
        nc.sync.dma_start(altp_sb[:], altp_d[:])
        one_sb = pool.tile([1, 1], f32r, tag="one")
        nc.sync.dma_start(one_sb[:], one_d[:])
        mhalf_sb = pool.tile([1, 1], f32r, tag="mhalf")
        nc.sync.dma_start(mhalf_sb[:], mhalf_d[:])
        mhrow_sb = pool.tile([1, NF], f32r, tag="mhrow")
        nc.sync.dma_start(mhrow_sb[:], mhrow_d[:])

        wqk_sb = pool.tile([128, H * 256], f32r, tag="wqk")
        for h in range(H):
            nc.sync.dma_start(wqk_sb[:, h * 256:(h + 1) * 256], wqk_d[h])
        wv_sb = pool.tile([128, H * E], f32r, tag="wv")
        for h in range(H):
            nc.sync.dma_start(wv_sb[:, h * E:(h + 1) * E], wv_d[h])
        wo32 = pool2.tile([128, H * E], f32, tag="vre")
        for h in range(H):
            nc.sync.dma_start(wo32[:, h * E:(h + 1) * E], wo_d[h])
        wo_sb = pool.tile([128, H * E], bf16, tag="wo")
        nc.scalar.copy(wo_sb[:], wo32[:])

        bqk_sb = pool.tile([1, 2 * H * E], f32, tag="bqk")
        nc.sync.dma_start(bqk_sb[:], bqk_d[:])
        bv_sb = pool.tile([128, H], f32, tag="bv")
        nc.sync.dma_start(bv_sb[:], bv_d[:])
        bo_sb = pool.tile([128, 1], f32, tag="bo")
        nc.sync.dma_start(bo_sb[:], bo_d[:])
        # big DFT matrices last, interleaved, so compute can chase the stream
        for i in range(NCH):
            nc.sync.dma_start(cs_sb[:, i * NF:(i + 1) * NF], cs_d[128 * i:128 * (i + 1), :])
            nc.sync.dma_start(sn_sb[:, i * NF:(i + 1) * NF], sn_d[128 * i:128 * (i + 1), :])

        # ---------------- fold ----------------
        xc = pool.tile([128, NCH * E], f32r, tag="gre0")
        xs = pool.tile([128, NCH * E], f32r, tag="gim0")
        nc.vector.tensor_add(xc[:], xt[:], xrt[:])
        nc.vector.tensor_sub(xs[:], xrt[:], xt[:])
        nc.vector.tensor_scalar_mul(xc[0:1, 0:E], xc[0:1, 0:E], 0.5)

        # ---------------- forward DFT ----------------
        hre = pool.tile([128, NF], f32r, tag="hre")
        him = pool.tile([128, NF], f32r, tag="him")
        hn = pool.tile([128, 1], f32r, tag="hn")
        for s in range(2):
            sl = slice(s * 512, (s + 1) * 512)
            hre_ps = ppool.tile([128, 512], f32, tag="bank")
            him_ps = ppool.tile([128, 512], f32, tag="bank")
            for a in range(NCH):
                nc.tensor.matmul(hre_ps[:], xc[:, a * E:(a + 1) * E],
                                 cs_sb[:, a * NF + s * 512: a * NF + (s + 1) * 512],
                                 start=(a == 0), stop=False)
            nc.tensor.matmul(hre_ps[:], xnyq[:], altf_sb[:, sl], start=False, stop=True)
            for a in range(NCH):
                nc.tensor.matmul(him_ps[:], xs[:, a * E:(a + 1) * E],
                                 sn_sb[:, a * NF + s * 512: a * NF + (s + 1) * 512],
                                 start=(a == 0), stop=(a == NCH - 1))
            nc.vector.tensor_copy(hre[:, sl], hre_ps[:])
            nc.vector.tensor_copy(him[:, sl], him_ps[:])
        hn_ps = prow.tile([128, 1], f32, tag="row")
        for a in range(NCH):
            nc.tensor.matmul(hn_ps[:], c32(xc[:, a * E:(a + 1) * E]), c32(altp_sb[:]),
                             start=(a == 0), stop=False)
        nc.tensor.matmul(hn_ps[:], c32(xnyq[:]), c32(one_sb[:]), start=False, stop=True)
        nc.vector.tensor_copy(hn[:], hn_ps[:])

        # ---------------- QK projections (transposed out) + products ----------------
        pre_r = pool.tile([128, NCH * E], f32r, tag="gre1")
        pim_r = pool.tile([128, NCH * E], f32r, tag="gim1")
        pre_t = pool.tile([128, NCH * E], f32, tag="scr_a")
        pim_t = pool.tile([128, NCH * E], f32, tag="scr_b")
        qn_row = pool.tile([1, E], f32, tag="qnrow")     # per-h rows accumulated into pn
        kn_row = pool.tile([1, E], f32, tag="knrow")
        pn_row = pool.tile([1, E], f32, tag="pnrow")
        pn_row_r = pool.tile([1, E], f32r, tag="pnrowr")
        pnw = pool.tile([1, E], f32, tag="pnw")

        for h in range(H):
            qkt_c = []
            for c in range(2):
                qkt = pool2.tile([128, NCH * 256], f32, tag="qkt")  # (j, qk, e')
                qkt_c.append(qkt)
                hsrc = hre if c == 0 else him
                for g in range(4):  # groups of 2 chunks -> one [128,512] psum
                    qk_ps = ppool.tile([128, 512], f32, tag="bank")
                    for jj in range(2):
                        j = g * 2 + jj
                        nc.tensor.matmul(qk_ps[:, jj * 256:(jj + 1) * 256],
                                         hsrc[:, j * 128:(j + 1) * 128],
                                         wqk_sb[:, h * 256:(h + 1) * 256],
                                         start=True, stop=True)
                    nc.scalar.copy(qkt[:, g * 512:(g + 1) * 512], qk_ps[:])
            # DC bias: f=0 is chunk0,row0 -> partition 0, cols (j=0, q|k, :)
            nc.vector.tensor_add(qkt_c[0][0:1, 0:128], qkt_c[0][0:1, 0:128], bqk_sb[0:1, h * E:(h + 1) * E])
            nc.vector.tensor_add(qkt_c[0][0:1, 128:256], qkt_c[0][0:1, 128:256], bqk_sb[0:1, H * E + h * E:H * E + (h + 1) * E])

            view_re = qkt_c[0][:].rearrange("p (j k e) -> k p j e", j=NCH, k=2)
            view_im = qkt_c[1][:].rearrange("p (j k e) -> k p j e", j=NCH, k=2)
            qr, kr = view_re[0], view_re[1]
            qi, ki = view_im[0], view_im[1]
            r3 = lambda ap: ap.rearrange("p (j e) -> p j e", j=NCH)

            eng_a = nc.vector
            eng_b = nc.gpsimd if h % 2 == 1 else nc.vector
            t1 = pool2.tile([128, NCH * E], f32, tag="ptmp")
            eng_a.tensor_tensor(r3(t1[:]), qr, kr, AL.mult)
            t2 = pool2.tile([128, NCH * E], f32, tag="ptmp")
            eng_a.tensor_tensor(r3(t2[:]), qi, ki, AL.mult)
            t3 = pool2.tile([128, NCH * E], f32, tag="ptmq")
            eng_b.tensor_tensor(r3(t3[:]), qi, kr, AL.mult)
            t4 = pool2.tile([128, NCH * E], f32, tag="ptmq")
            eng_b.tensor_tensor(r3(t4[:]), qr, ki, AL.mult)
            if h == 0:
                eng_a.tensor_add(pre_t[:], t1[:], t2[:])
                eng_b.tensor_sub(pim_t[:], t3[:], t4[:])
            else:
                eng_a.tensor_add(pre_t[:], pre_t[:], t1[:])
                eng_a.tensor_add(pre_t[:], pre_t[:], t2[:])
                eng_b.tensor_add(pim_t[:], pim_t[:], t3[:])
                eng_b.tensor_sub(pim_t[:], pim_t[:], t4[:])
            if h == H - 1:
                s_scale = 2.0 / (T * H)
                nc.vector.tensor_scalar_mul(pre_r[:], pre_t[:], s_scale)
                nc.gpsimd.tensor_scalar_mul(pim_r[:], pim_t[:], s_scale)

            # Nyquist rows: qn_row = hn^T @ wq_h ; kn_row = hn^T @ wk_h
            r_ps = prow.tile([1, 256], f32, tag="row")
            nc.tensor.matmul(r_ps[:], c32(hn[:]), c32(wqk_sb[:, h * 256:(h + 1) * 256]), start=True, stop=True)
            nc.scalar.copy(qn_row[:], r_ps[:, 0:128])
            nc.scalar.copy(kn_row[:], r_ps[:, 128:256])
            if h == 0:
                nc.vector.scalar_tensor_tensor(pn_row[:], qn_row[:], 1.0 / (T * H), kn_row[:], AL.mult, AL.mult)
            else:
                nc.vector.scalar_tensor_tensor(pnw[:], qn_row[:], 1.0 / (T * H), kn_row[:], AL.mult, AL.mult)
                dst = pn_row_r if h == H - 1 else pn_row
                nc.vector.tensor_add(dst[:], pn_row[:], pnw[:])

        # ---------------- acm inverse ----------------
        eo_sb = pool2.tile([128, T], f32, tag="qkt")
        for s in range(2):
            sl = slice(s * 512, (s + 1) * 512)
            e_ps = ppool.tile([128, 512], f32, tag="bank")
            o_ps = ppool.tile([128, 512], f32, tag="bank")
            for j in range(NCH):
                nc.tensor.matmul(e_ps[:], pre_r[:, j * E:(j + 1) * E],
                                 cs_sb[:, j * NF + s * 512: j * NF + (s + 1) * 512],
                                 start=(j == 0), stop=False)
            nc.tensor.matmul(e_ps[:], pn_row_r[:], altf_sb[:, sl], start=False, stop=True)
            for j in range(NCH):
                nc.tensor.matmul(o_ps[:], pim_r[:, j * E:(j + 1) * E],
                                 sn_sb[:, j * NF + s * 512: j * NF + (s + 1) * 512],
                                 start=(j == 0), stop=(j == NCH - 1))
            e_sb = pool2.tile([128, 512], f32, tag="ecp")
            nc.scalar.copy(e_sb[:], e_ps[:])
            nc.vector.tensor_sub(eo_sb[:, sl], e_sb[:], o_ps[:])
            if s == 0:
                nc.vector.tensor_add(eo_sb[:, NF + 1:NF + 512], e_sb[:, 1:], o_ps[:, 1:])
            else:
                nc.vector.tensor_add(eo_sb[:, NF + 512:2 * NF], e_sb[:], o_ps[:])
        a1024_ps = prow.tile([128, 1], f32, tag="row")
        for j in range(NCH):
            nc.tensor.matmul(a1024_ps[:], c32(pre_r[:, j * E:(j + 1) * E]), c32(altp_sb[:]),
                             start=(j == 0), stop=False)
        nc.tensor.matmul(a1024_ps[:], c32(pn_row_r[:]), c32(one_sb[:]), start=False, stop=True)
        nc.scalar.copy(eo_sb[:, NF:NF + 1], a1024_ps[:])

        # ---------------- top-k ----------------
        vals = pool.tile([128, 24], f32, tag="vals")
        nc.vector.max(vals[:, 0:8], eo_sb[:])
        nc.vector.match_replace(eo_sb[:], vals[:, 0:8], eo_sb[:], -1e30)
        nc.vector.max(vals[:, 8:16], eo_sb[:])
        nc.vector.match_replace(eo_sb[:], vals[:, 8:16], eo_sb[:], -1e30)
        nc.vector.max(vals[:, 16:24], eo_sb[:])
        idx8 = pool.tile([128, 8], u32, tag="idx8")
        nc.vector.max_index(idx8[:], vals[:, 16:24], eo_sb[:])

        c_i = pool.tile([128, 1], i32, tag="ci")
        nc.vector.tensor_copy(c_i[:], idx8[:, 5:6].bitcast(i32))
        c_neg = pool.tile([128, 1], i32, tag="cneg")
        nc.vector.tensor_scalar(c_neg[:], c_i[:], -1, 3072, AL.mult, AL.add)
        mask = pool.tile([128, 1], i32, tag="mask")
        nc.vector.tensor_scalar(mask[:], c_i[:], 1024, None, AL.is_gt)
        d_i = pool.tile([128, 1], i32, tag="di")
        nc.vector.select(d_i[:], mask[:], c_neg[:], c_i[:])
        d_f = pool.tile([128, 1], f32, tag="df")
        nc.vector.tensor_copy(d_f[:], d_i[:])

        negv0 = pool.tile([128, 1], f32, tag="negv0")
        nc.vector.tensor_scalar_mul(negv0[:], vals[:, 0:1], -1.0)
        expv = pool.tile([128, 24], f32, tag="expv")
        nc.scalar.activation(expv[:, 0:22], vals[:, 0:22], AF.Exp, bias=negv0[:])
        den = pool.tile([128, 1], f32, tag="den")
        nc.vector.tensor_reduce(den[:], expv[:, 0:22], AX.X, AL.add)
        rden = pool.tile([128, 1], f32, tag="rden")
        nc.vector.reciprocal(rden[:], den[:])
        wgt = pool.tile([128, 1], f32, tag="wgt")
        nc.vector.tensor_mul(wgt[:], expv[:, 21:22], rden[:])

        # ---------------- phases ----------------
        negpi = pool.tile([128, 1], f32, tag="negpi")
        nc.gpsimd.memset(negpi[:], float(-np.pi))
        io_i = pool.tile([128, NF], i32, tag="scr_a")
        nc.gpsimd.iota(io_i[:], pattern=[[1, NF]], base=0, channel_multiplier=0)
        io_f = pool.tile([128, NF], f32, tag="scr_b")
        nc.vector.tensor_copy(io_f[:], io_i[:])
        cw = pool.tile([128, NF], bf16, tag="cw")
        sw = pool.tile([128, NF], bf16, tag="sw")
        # -Sin(pi/1024 * ((f*d + off)&2047) - pi) = sin/cos(2*pi*f*d/2048); the
        # leading minus is folded into the -2/T weight scale below.
        for off, dst in ((0.0, sw), (512.0, cw)):
            mf = pool2.tile([128, NF], f32, tag="ptmp")
            if off == 0.0:
                nc.vector.tensor_scalar(mf[:], io_f[:], d_f[:], None, AL.mult)
            else:
                nc.vector.tensor_scalar(mf[:], io_f[:], d_f[:], off, AL.mult, AL.add)
            mi = pool2.tile([128, NF], i32, tag="ptmp")
            nc.vector.tensor_copy(mi[:], mf[:])
            nc.vector.tensor_scalar(mi[:], mi[:], 2047, None, AL.bitwise_and)
            mf2 = pool2.tile([128, NF], f32, tag="ptmp")
            nc.vector.tensor_copy(mf2[:], mi[:])
            ph = pool2.tile([128, NF], f32, tag="ptmp")
            nc.scalar.activation(ph[:], mf2[:], AF.Sin,
                                 scale=float(np.pi / 1024.0), bias=negpi[:])
            nc.vector.tensor_scalar(dst[:], ph[:], wgt[:], -2.0 / T, AL.mult, AL.mult)
        swn = pool.tile([128, NF], bf16, tag="swn")
        nc.vector.tensor_scalar_mul(swn[:], sw[:], -1.0)
        # nyquist scale: (1-2*(d&1)) * wgt / T
        par_i = pool.tile([128, 1], i32, tag="par")
        nc.vector.tensor_scalar(par_i[:], d_i[:], 1, None, AL.bitwise_and)
        parf = pool.tile([128, 1], f32, tag="parf")
        nc.vector.tensor_copy(parf[:], par_i[:])
        nc.vector.tensor_scalar(parf[:], parf[:], -2.0, 1.0, AL.mult, AL.add)
        nys = pool.tile([128, 1], f32, tag="nys")
        nc.vector.tensor_scalar(nys[:], parf[:], wgt[:], 1.0 / T, AL.mult, AL.mult)

        # ---------------- V spectra + phase multiply (all h) ----------------
        gre_t = {}
        gim_t = {}
        gn_cols = pool.tile([128, H], bf16, tag="gncols")
        for h in range(H):
            vre = pool2.tile([128, NF], bf16, tag="vre")
            vim = pool2.tile([128, NF], bf16, tag="vim")
            for c in range(2):
                hsrc = hre if c == 0 else him
                dst = vre if c == 0 else vim
                for s in range(2):
                    v_ps = ppool.tile([128, 512], f32, tag="bank")
                    nc.tensor.matmul(v_ps[:], wv_sb[:, h * E:(h + 1) * E],
                                     hsrc[:, s * 512:(s + 1) * 512], start=True, stop=True)
                    if c == 0 and s == 0:
                        nc.vector.tensor_add(v_ps[:, 0:1], v_ps[:, 0:1], bv_sb[:, h:h + 1])
                    nc.scalar.copy(dst[:, s * 512:(s + 1) * 512], v_ps[:])
            vn_ps = prow.tile([128, 1], f32, tag="row")
            nc.tensor.matmul(vn_ps[:], c32(wv_sb[:, h * E:(h + 1) * E]), c32(hn[:]), start=True, stop=True)
            vn_col = pool.tile([128, 1], f32, tag=f"vn{h}")
            nc.scalar.copy(vn_col[:], vn_ps[:])
            nc.vector.tensor_mul(gn_cols[:, h:h + 1], vn_col[:], nys[:])

            m1 = pool.tile([128, NF], bf16, tag=f"gre{h}")
            m2 = pool.tile([128, NF], bf16, tag=f"gren{h}")
            m3 = pool.tile([128, NF], bf16, tag=f"gim{h}")
            m4 = pool.tile([128, NF], bf16, tag=f"gimn{h}")
            nc.vector.tensor_mul(m1[:], vre[:], cw[:])
            nc.vector.tensor_mul(m2[:], vim[:], swn[:])
            nc.vector.tensor_mul(m3[:], vre[:], sw[:])
            nc.vector.tensor_mul(m4[:], vim[:], cw[:])
            gre_t[h] = (m1, m2)
            gim_t[h] = (m3, m4)

        # ---------------- output projection ----------------
        of_re = pool.tile([128, NCH * E], f32r, tag="ofre")
        of_im = pool.tile([128, NCH * E], f32r, tag="ofim")
        for j in range(NCH):
            for c in range(2):
                of_ps = ppool.tile([128, 512], f32, tag="bank")
                gt = gre_t if c == 0 else gim_t
                for h in range(H):
                    ga, gb = gt[h]
                    nc.tensor.matmul(of_ps[:, 0:128], ga[:, j * 128:(j + 1) * 128],
                                     wo_sb[:, h * E:(h + 1) * E],
                                     start=(h == 0), stop=False)
                    nc.tensor.matmul(of_ps[:, 0:128], gb[:, j * 128:(j + 1) * 128],
                                     wo_sb[:, h * E:(h + 1) * E],
                                     start=False, stop=(h == H - 1))
                dst = of_re if c == 0 else of_im
                nc.vector.tensor_copy(dst[:, j * E:(j + 1) * E], of_ps[:, 0:128])
        ofn_ps = prow.tile([1, E], f32, tag="row")
        for h in range(H):
            nc.tensor.matmul(ofn_ps[:], gn_cols[:, h:h + 1], wo_sb[:, h * E:(h + 1) * E],
                             start=(h == 0), stop=(h == H - 1))
        ofn_row = pool.tile([1, E], f32r, tag="ofnrow")
        nc.vector.tensor_copy(ofn_row[:], ofn_ps[:])

        # ---------------- final inverse ----------------
        for s in range(2):
            sl = slice(s * 512, (s + 1) * 512)
            ep_ps = ppool.tile([128, 512], f32, tag="bank")
            op_ps = ppool.tile([128, 512], f32, tag="bank")
            for j in range(NCH):
                nc.tensor.matmul(ep_ps[:], of_re[:, j * E:(j + 1) * E],
                                 cs_sb[:, j * NF + s * 512: j * NF + (s + 1) * 512],
                                 start=(j == 0), stop=False)
            nc.tensor.matmul(ep_ps[:], ofn_row[:], altf_sb[:, sl], start=False, stop=False)
            # -0.5 * Of_re[0, f''] constant-in-t correction (f=0 weight fix)
            nc.tensor.matmul(ep_ps[:], of_re[0:1, 0:E], mhrow_sb[:, sl], start=False, stop=True)
            for j in range(NCH):
                nc.tensor.matmul(op_ps[:], of_im[:, j * E:(j + 1) * E],
                                 sn_sb[:, j * NF + s * 512: j * NF + (s + 1) * 512],
                                 start=(j == 0), stop=(j == NCH - 1))
            ep_sb = pool2.tile([128, 512], f32, tag="ecp")
            nc.scalar.copy(ep_sb[:], ep_ps[:])
            out_lo = pool2.tile([128, 512], f32, tag="outlo")
            out_hi = pool2.tile([128, 512], f32, tag="outlo")
            nc.vector.scalar_tensor_tensor(out_lo[:], ep_sb[:], bo_sb[:], op_ps[:], AL.add, AL.subtract)
            nc.vector.scalar_tensor_tensor(out_hi[:], ep_sb[:], bo_sb[:], op_ps[:], AL.add, AL.add)
            nc.sync.dma_start(lo_d[:, sl], out_lo[:])
            nc.sync.dma_start(hi_d[:, sl], out_hi[:])
        # t = 1024 row
        o1_ps = prow.tile([128, 1], f32, tag="row")
        for j in range(NCH):
            nc.tensor.matmul(o1_ps[:], c32(of_re[:, j * E:(j + 1) * E]), c32(altp_sb[:]),
                             start=(j == 0), stop=False)
        nc.tensor.matmul(o1_ps[:], c32(ofn_row[:]), c32(one_sb[:]), start=False, stop=False)
        nc.tensor.matmul(o1_ps[:], c32(of_re[0:1, 0:E]), c32(mhalf_sb[:]), start=False, stop=True)
        o1_sb = pool.tile([128, 1], f32, tag="o1sb")
        nc.vector.tensor_scalar(o1_sb[:], o1_ps[:], bo_sb[:], None, AL.add)
        nc.sync.dma_start(o1024_d[:], o1_sb[:])

    nc.compile()
    return nc


def _get_nc():
    if "nc" not in _CACHE:
        _wire_ntff_hook()
        _CACHE["nc"] = _build()
    return _CACHE["nc"]


def kernel(hidden_states, wq, bq, wk, bk, wv, bv, wo, bo):
    global LAST_EXEC_NS
    nc = _get_nc()
    consts = _CACHE.setdefault("consts", _host_consts())

    hs = np.ascontiguousarray(hidden_states, dtype=np.float32)
    wqk = np.concatenate([wq.transpose(2, 0, 1), wk.transpose(2, 0, 1)], axis=2)  # [H, E, 256]
    wv_h = np.ascontiguousarray(wv.transpose(2, 0, 1), dtype=np.float32)          # [H, E, E]
    wo_h = np.ascontiguousarray(wo.transpose(1, 0, 2), dtype=np.float32)          # [H, E, E]
    bqk = np.concatenate([(T * bq.T).reshape(-1), (T * bk.T).reshape(-1)])[None, :].astype(np.float32)  # [1, 2*H*E]
    bv_s = np.ascontiguousarray(T * bv, dtype=np.float32)                         # [E, H]
    bo_c = np.ascontiguousarray(bo, dtype=np.float32)[:, None]                    # [E, 1]

    in_maps = []
    for b in range(B):
        x = hs[b]
        xr = np.concatenate([x[0:1], x[:0:-1]])[:NF]
        in_maps.append({
            "x": x, "xr": np.ascontiguousarray(xr),
            "cs": consts["cs"], "sn": consts["sn"], "altf": consts["altf"],
            "altp": consts["altp"], "one": consts["one"], "mhalf": consts["mhalf"],
            "mhrow": consts["mhrow"],
            "wqk": np.ascontiguousarray(wqk, dtype=np.float32), "wv": wv_h,
            "wo": wo_h, "bqk": bqk, "bv": bv_s, "bo": bo_c,
        })

    trace = bool(int(os.environ.get("BASS_KERNEL_TRACE", "0")))
    res = run_bass_kernel_spmd(nc, in_maps, core_ids=list(range(B)), trace=trace)
    LAST_EXEC_NS = res.exec_time_ns
    _CACHE["last_res"] = res

    out = np.empty((B, T, E), dtype=np.float32)
    for b in range(B):
        r = res.results[b]
        out[b, 0:NF] = r["out_lo"].T
        out[b, NF] = r["out_1024"][:, 0]
        out[b, NF + 1:] = r["out_hi"][:, 1:NF][:, ::-1].T
    return out


# revision 22
# speedup vs baseline: 1.1512x; 1.1498x over previous
"""AutoCorrelation (Autoformer-style) Trainium2 Bass kernel.

Sharding: data-parallel over batch — 8 batch elements -> 8 NeuronCores, no
collectives. Each core computes its [2048, 128] output slice independently.

Algorithm (validated against the reference in fp64 numpy to ~1e-7):
  * Single folded real-DFT matrix pair C,S = cos/sin(2*pi*i*j/2048) [1024x1024];
    symmetry of the matrix serves forward rfft and both inverse transforms.
  * Q/K/V are obtained by projecting the hidden spectrum (DFT linearity);
    biases enter only the DC bin (+T*b) — the Nyquist bin is carried separately.
  * acm = irfft(mean_h Qf conj(Kf)) via matmuls. Interior rfft weights (2/T)
    are applied uniformly; the resulting f=0 offset is constant per channel and
    cancels in both top-k ordering and softmax.
  * top-22 per channel: 3 rounds of VectorE max8 + match_replace; the 22nd
    delay from max_index; acm is kept in a permuted layout (lower half E-O,
    upper half E+O) and the delay index is remapped arithmetically.
  * The circular roll of V is a per-channel phase multiply in the frequency
    domain (shift theorem); softmax weight and 2/T fold into the phase tables.
  * float32r (full-rate fp32 matmul mode) on the accuracy-critical path,
    bf16 on the V/output path; elementwise complex products in fp32 split
    between VectorE and GpSimd.
"""
import os
import sys
import types
from contextlib import ExitStack

sys.path.insert(0, "/opt/trn_rl_repo")

import numpy as np

import concourse.bass as bass
import concourse.mybir as mybir
from concourse import bacc
from concourse.tile import TileContext
from concourse.bass_utils import run_bass_kernel_spmd

B, T, E, H = 8, 2048, 128, 4
NF = 1024
NCH = 8
AL = mybir.AluOpType
DT = mybir.dt
AF = mybir.ActivationFunctionType
AX = mybir.AxisListType

_CACHE = {}
LAST_EXEC_NS = None


def _wire_ntff_hook():
    if "antenv.axon_hooks" in sys.modules:
        return
    try:
        mod = types.ModuleType("antenv.axon_hooks")
        _h = [None]
        mod.set_axon_ntff_profile_hook = lambda h: _h.__setitem__(0, h)
        mod.get_axon_ntff_profile_hook = lambda: _h[0]
        sys.modules["antenv.axon_hooks"] = mod
        import antenv
        antenv.axon_hooks = mod
        from trn_agent_boot.trn_boot import _ntff_profile_via_ctypes
        mod.set_axon_ntff_profile_hook(_ntff_profile_via_ctypes("/opt/axon/libaxon_pjrt.so"))
    except Exception:
        pass


def _host_consts():
    i = np.arange(NF, dtype=np.float64)
    ang = np.outer(i, i) * (2.0 * np.pi / T)
    return {
        "cs": np.cos(ang).astype(np.float32),
        "sn": np.sin(ang).astype(np.float32),
        "altf": ((-1.0) ** np.arange(NF)).astype(np.float32)[None, :],
        "altp": ((-1.0) ** np.arange(128)).astype(np.float32)[:, None],
        "one": np.ones((1, 1), np.float32),
        "mhalf": np.full((1, 1), -0.5, np.float32),
        "mhrow": np.full((1, NF), -0.5, np.float32),
    }


def _build():
    nc = bacc.Bacc("TRN2", target_bir_lowering=False, debug=False, num_devices=1)
    f32, f32r, bf16, i32, u32 = DT.float32, DT.float32r, DT.bfloat16, DT.int32, DT.uint32

    x_d = nc.dram_tensor("x", [T, E], f32r, kind="ExternalInput")
    xr_d = nc.dram_tensor("xr", [NF, E], f32r, kind="ExternalInput")
    cs_d = nc.dram_tensor("cs", [NF, NF], f32r, kind="ExternalInput")
    sn_d = nc.dram_tensor("sn", [NF, NF], f32r, kind="ExternalInput")
    altf_d = nc.dram_tensor("altf", [1, NF], f32r, kind="ExternalInput")
    altp_d = nc.dram_tensor("altp", [128, 1], f32r, kind="ExternalInput")
    one_d = nc.dram_tensor("one", [1, 1], f32r, kind="ExternalInput")
    mhalf_d = nc.dram_tensor("mhalf", [1, 1], f32r, kind="ExternalInput")
    mhrow_d = nc.dram_tensor("mhrow", [1, NF], f32r, kind="ExternalInput")
    wqk_d = nc.dram_tensor("wqk", [H, E, 256], f32r, kind="ExternalInput")
    wv_d = nc.dram_tensor("wv", [H, E, E], f32r, kind="ExternalInput")
    wo_d = nc.dram_tensor("wo", [H, E, E], f32, kind="ExternalInput")
    bqk_d = nc.dram_tensor("bqk", [1, 2 * H * E], f32, kind="ExternalInput")  # T*bq | T*bk (h-major)
    bv_d = nc.dram_tensor("bv", [E, H], f32, kind="ExternalInput")        # T*bv
    bo_d = nc.dram_tensor("bo", [E, 1], f32, kind="ExternalInput")
    lo_d = nc.dram_tensor("out_lo", [E, NF], f32, kind="ExternalOutput")
    hi_d = nc.dram_tensor("out_hi", [E, NF], f32, kind="ExternalOutput")
    o1024_d = nc.dram_tensor("out_1024", [E, 1], f32, kind="ExternalOutput")

    c32 = lambda ap: ap.bitcast(DT.float32)
    with TileContext(nc) as tc, ExitStack() as ctx:
        pool = ctx.enter_context(tc.tile_pool(name="main", bufs=1))
        pool2 = ctx.enter_context(tc.tile_pool(name="rot", bufs=2))
        ppool = ctx.enter_context(tc.tile_pool(name="ps", bufs=6, space="PSUM"))
        prow = ctx.enter_context(tc.tile_pool(name="psrow", bufs=2, space="PSUM"))

        # ---------------- loads ----------------
        cs_sb = pool.tile([128, NCH * NF], f32r, tag="cs")
        sn_sb = pool.tile([128, NCH * NF], f32r, tag="sn")

        xt = pool.tile([128, NCH * E], f32r, tag="scr_a")
        xrt = pool.tile([128, NCH * E], f32r, tag="scr_b")
        for a in range(NCH):
            nc.sync.dma_start(xt[:, a * E:(a + 1) * E], x_d[128 * a:128 * (a + 1), :])
            nc.sync.dma_start(xrt[:, a * E:(a + 1) * E], xr_d[128 * a:128 * (a + 1), :])
        xnyq = pool.tile([1, E], f32r, tag="xnyq")
        nc.sync.dma_start(xnyq[:], x_d[NF:NF + 1, :])

        altf_sb = pool.tile([1, NF], f32r, tag="altf")
        nc.sync.dma_start(altf_sb[:], altf_d[:])
        altp_sb = pool.tile([128, 1], f32r, tag="altp")
        nc.sync.dma_start(altp_sb[:], altp_d[:])
        one_sb = pool.tile([1, 1], f32r, tag="one")
        nc.sync.dma_start(one_sb[:], one_d[:])
        mhalf_sb = pool.tile([1, 1], f32r, tag="mhalf")
        nc.sync.dma_start(mhalf_sb[:], mhalf_d[:])
        mhrow_sb = pool.tile([1, NF], f32r, tag="mhrow")
        nc.sync.dma_start(mhrow_sb[:], mhrow_d[:])

        wqk_sb = pool.tile([128, H * 256], f32r, tag="wqk")
        for h in range(H):
            nc.sync.dma_start(wqk_sb[:, h * 256:(h + 1) * 256], wqk_d[h])
        wv_sb = pool.tile([128, H * E], f32r, tag="wv")
        for h in range(H):
            nc.sync.dma_start(wv_sb[:, h * E:(h + 1) * E], wv_d[h])
        wo32 = pool2.tile([128, H * E], f32, tag="vre")
        for h in range(H):
            nc.sync.dma_start(wo32[:, h * E:(h + 1) * E], wo_d[h])
        wo_sb = pool.tile([128, H * E], bf16, tag="wo")
        nc.scalar.copy(wo_sb[:], wo32[:])

        bqk_sb = pool.tile([1, 2 * H * E], f32, tag="bqk")
        nc.sync.dma_start(bqk_sb[:], bqk_d[:])
        bv_sb = pool.tile([128, H], f32, tag="bv")
        nc.sync.dma_start(bv_sb[:], bv_d[:])
        bo_sb = pool.tile([128, 1], f32, tag="bo")
        nc.sync.dma_start(bo_sb[:], bo_d[:])
        # big DFT matrices last, interleaved, so compute can chase the stream
        for i in range(NCH):
            nc.sync.dma_start(cs_sb[:, i * NF:(i + 1) * NF], cs_d[128 * i:128 * (i + 1), :])
            nc.sync.dma_start(sn_sb[:, i * NF:(i + 1) * NF], sn_d[128 * i:128 * (i + 1), :])

        # ---------------- fold ----------------
        xc = pool.tile([128, NCH * E], f32r, tag="gre0")
        xs = pool.tile([128, NCH * E], f32r, tag="gim0")
        nc.vector.tensor_add(xc[:], xt[:], xrt[:])
        nc.vector.tensor_sub(xs[:], xrt[:], xt[:])
        nc.vector.tensor_scalar_mul(xc[0:1, 0:E], xc[0:1, 0:E], 0.5)

        # ---------------- forward DFT ----------------
        hre = pool.tile([128, NF], f32r, tag="hre")
        him = pool.tile([128, NF], f32r, tag="him")
        hn = pool.tile([128, 1], f32r, tag="hn")
        for s in range(2):
            sl = slice(s * 512, (s + 1) * 512)
            hre_ps = ppool.tile([128, 512], f32, tag="bank")
            him_ps = ppool.tile([128, 512], f32, tag="bank")
            for a in range(NCH):
                nc.tensor.matmul(hre_ps[:], xc[:, a * E:(a + 1) * E],
                                 cs_sb[:, a * NF + s * 512: a * NF + (s + 1) * 512],
                                 start=(a == 0), stop=False)
            nc.tensor.matmul(hre_ps[:], xnyq[:], altf_sb[:, sl], start=False, stop=True)
            for a in range(NCH):
                nc.tensor.matmul(him_ps[:], xs[:, a * E:(a + 1) * E],
                                 sn_sb[:, a * NF + s * 512: a * NF + (s + 1) * 512],
                                 start=(a == 0), stop=(a == NCH - 1))
            nc.vector.tensor_copy(hre[:, sl], hre_ps[:])
            nc.vector.tensor_copy(him[:, sl], him_ps[:])
        hn_ps = prow.tile([128, 1], f32, tag="row")
        for a in range(NCH):
            nc.tensor.matmul(hn_ps[:], c32(xc[:, a * E:(a + 1) * E]), c32(altp_sb[:]),
                             start=(a == 0), stop=False)
        nc.tensor.matmul(hn_ps[:], c32(xnyq[:]), c32(one_sb[:]), start=False, stop=True)
        nc.vector.tensor_copy(hn[:], hn_ps[:])

        # ---------------- QK projections (transposed out) + products ----------------
        pre_r = pool.tile([128, NCH * E], f32r, tag="gre1")
        pim_r = pool.tile([128, NCH * E], f32r, tag="gim1")
        pre_t = pool.tile([128, NCH * E], f32, tag="scr_a")
        pim_t = pool.tile([128, NCH * E], f32, tag="scr_b")
        qn_row = pool.tile([1, E], f32, tag="qnrow")     # per-h rows accumulated into pn
        kn_row = pool.tile([1, E], f32, tag="knrow")
        pn_row = pool.tile([1, E], f32, tag="pnrow")
        pn_row_r = pool.tile([1, E], f32r, tag="pnrowr")
        pnw = pool.tile([1, E], f32, tag="pnw")

        for h in range(H):
            qkt_c = []
            for c in range(2):
                qkt = pool2.tile([128, NCH * 256], f32, tag="qkt")  # (j, qk, e')
                qkt_c.append(qkt)
                hsrc = hre if c == 0 else him
                for g in range(4):  # groups of 2 chunks -> one [128,512] psum
                    qk_ps = ppool.tile([128, 512], f32, tag="bank")
                    for jj in range(2):
                        j = g * 2 + jj
                        nc.tensor.matmul(qk_ps[:, jj * 256:(jj + 1) * 256],
                                         hsrc[:, j * 128:(j + 1) * 128],
                                         wqk_sb[:, h * 256:(h + 1) * 256],
                                         start=True, stop=True)
                    nc.scalar.copy(qkt[:, g * 512:(g + 1) * 512], qk_ps[:])
            # DC bias: f=0 is chunk0,row0 -> partition 0, cols (j=0, q|k, :)
            nc.vector.tensor_add(qkt_c[0][0:1, 0:128], qkt_c[0][0:1, 0:128], bqk_sb[0:1, h * E:(h + 1) * E])
            nc.vector.tensor_add(qkt_c[0][0:1, 128:256], qkt_c[0][0:1, 128:256], bqk_sb[0:1, H * E + h * E:H * E + (h + 1) * E])

            view_re = qkt_c[0][:].rearrange("p (j k e) -> k p j e", j=NCH, k=2)
            view_im = qkt_c[1][:].rearrange("p (j k e) -> k p j e", j=NCH, k=2)
            qr, kr = view_re[0], view_re[1]
            qi, ki = view_im[0], view_im[1]
            r3 = lambda ap: ap.rearrange("p (j e) -> p j e", j=NCH)

            eng_b = nc.gpsimd if h >= 1 else nc.vector
            t1 = pool2.tile([128, NCH * E], f32, tag="ptmp")
            nc.vector.tensor_tensor(r3(t1[:]), qr, kr, AL.mult)
            t2 = pool2.tile([128, NCH * E], f32, tag="ptmp")
            nc.vector.tensor_tensor(r3(t2[:]), qi, ki, AL.mult)
            t3 = pool2.tile([128, NCH * E], f32, tag="ptmq")
            eng_b.tensor_tensor(r3(t3[:]), qi, kr, AL.mult)
            t4 = pool2.tile([128, NCH * E], f32, tag="ptmq")
            eng_b.tensor_tensor(r3(t4[:]), qr, ki, AL.mult)
            if h == 0:
                nc.vector.tensor_add(pre_t[:], t1[:], t2[:])
                nc.vector.tensor_sub(pim_t[:], t3[:], t4[:])
            else:
                nc.vector.tensor_add(pre_t[:], pre_t[:], t1[:])
                nc.vector.tensor_add(pre_t[:], pre_t[:], t2[:])
                nc.vector.tensor_add(pim_t[:], pim_t[:], t3[:])
                nc.vector.tensor_sub(pim_t[:], pim_t[:], t4[:])
            if h == H - 1:
                s_scale = 2.0 / (T * H)
                nc.vector.tensor_scalar_mul(pre_r[:], pre_t[:], s_scale)
                nc.gpsimd.tensor_scalar_mul(pim_r[:], pim_t[:], s_scale)

            # Nyquist rows: qn_row = hn^T @ wq_h ; kn_row = hn^T @ wk_h
            r_ps = prow.tile([1, 256], f32, tag="row")
            nc.tensor.matmul(r_ps[:], c32(hn[:]), c32(wqk_sb[:, h * 256:(h + 1) * 256]), start=True, stop=True)
            nc.scalar.copy(qn_row[:], r_ps[:, 0:128])
            nc.scalar.copy(kn_row[:], r_ps[:, 128:256])
            if h == 0:
                nc.vector.scalar_tensor_tensor(pn_row[:], qn_row[:], 1.0 / (T * H), kn_row[:], AL.mult, AL.mult)
            else:
                nc.vector.scalar_tensor_tensor(pnw[:], qn_row[:], 1.0 / (T * H), kn_row[:], AL.mult, AL.mult)
                dst = pn_row_r if h == H - 1 else pn_row
                nc.vector.tensor_add(dst[:], pn_row[:], pnw[:])

        # ---------------- acm inverse ----------------
        eo_sb = pool2.tile([128, T], f32, tag="qkt")
        for s in range(2):
            sl = slice(s * 512, (s + 1) * 512)
            e_ps = ppool.tile([128, 512], f32, tag="bank")
            o_ps = ppool.tile([128, 512], f32, tag="bank")
            for j in range(NCH):
                nc.tensor.matmul(e_ps[:], pre_r[:, j * E:(j + 1) * E],
                                 cs_sb[:, j * NF + s * 512: j * NF + (s + 1) * 512],
                                 start=(j == 0), stop=False)
            nc.tensor.matmul(e_ps[:], pn_row_r[:], altf_sb[:, sl], start=False, stop=True)
            for j in range(NCH):
                nc.tensor.matmul(o_ps[:], pim_r[:, j * E:(j + 1) * E],
                                 sn_sb[:, j * NF + s * 512: j * NF + (s + 1) * 512],
                                 start=(j == 0), stop=(j == NCH - 1))
            e_sb = pool2.tile([128, 512], f32, tag="ecp")
            nc.scalar.copy(e_sb[:], e_ps[:])
            nc.vector.tensor_sub(eo_sb[:, sl], e_sb[:], o_ps[:])
            if s == 0:
                nc.vector.tensor_add(eo_sb[:, NF + 1:NF + 512], e_sb[:, 1:], o_ps[:, 1:])
            else:
                nc.vector.tensor_add(eo_sb[:, NF + 512:2 * NF], e_sb[:], o_ps[:])
        a1024_ps = prow.tile([128, 1], f32, tag="row")
        for j in range(NCH):
            nc.tensor.matmul(a1024_ps[:], c32(pre_r[:, j * E:(j + 1) * E]), c32(altp_sb[:]),
                             start=(j == 0), stop=False)
        nc.tensor.matmul(a1024_ps[:], c32(pn_row_r[:]), c32(one_sb[:]), start=False, stop=True)
        nc.scalar.copy(eo_sb[:, NF:NF + 1], a1024_ps[:])

        # ---------------- V spectra h0/h1 (overlaps top-k on PE) ----------------
        vn_cols = pool.tile([128, H], f32, tag="vncols")
        vhalf = {}

        def v_proj(hlist, tile_ref):
            for h in hlist:
                for c in range(2):
                    hsrc = hre if c == 0 else him
                    dst_off = (2 * (h % 2) + c) * NF
                    for sv in range(2):
                        v_ps = ppool.tile([128, 512], f32, tag="bank")
                        nc.tensor.matmul(v_ps[:], wv_sb[:, h * E:(h + 1) * E],
                                         hsrc[:, sv * 512:(sv + 1) * 512], start=True, stop=True)
                        if c == 0 and sv == 0:
                            nc.vector.tensor_add(v_ps[:, 0:1], v_ps[:, 0:1], bv_sb[:, h:h + 1])
                        nc.scalar.copy(tile_ref[:, dst_off + sv * 512:dst_off + (sv + 1) * 512], v_ps[:])
                vn_ps = prow.tile([128, 1], f32, tag="row")
                nc.tensor.matmul(vn_ps[:], c32(wv_sb[:, h * E:(h + 1) * E]), c32(hn[:]), start=True, stop=True)
                nc.scalar.copy(vn_cols[:, h:h + 1], vn_ps[:])

        vqk_a = pool2.tile([128, H * NF], bf16, tag="qkt")  # (h01, c, f)
        vhalf[0] = vqk_a
        v_proj([0, 1], vqk_a)

        # ---------------- top-k ----------------
        vals = pool.tile([128, 24], f32, tag="vals")
        nc.vector.max(vals[:, 0:8], eo_sb[:])
        nc.vector.match_replace(eo_sb[:], vals[:, 0:8], eo_sb[:], -1e30)
        nc.vector.max(vals[:, 8:16], eo_sb[:])
        nc.vector.match_replace(eo_sb[:], vals[:, 8:16], eo_sb[:], -1e30)
        nc.vector.max(vals[:, 16:24], eo_sb[:])
        idx8 = pool.tile([128, 8], u32, tag="idx8")
        nc.vector.max_index(idx8[:], vals[:, 16:24], eo_sb[:])

        c_i = pool.tile([128, 1], i32, tag="ci")
        nc.vector.tensor_copy(c_i[:], idx8[:, 5:6].bitcast(i32))
        c_neg = pool.tile([128, 1], i32, tag="cneg")
        nc.vector.tensor_scalar(c_neg[:], c_i[:], -1, 3072, AL.mult, AL.add)
        mask = pool.tile([128, 1], i32, tag="mask")
        nc.vector.tensor_scalar(mask[:], c_i[:], 1024, None, AL.is_gt)
        d_i = pool.tile([128, 1], i32, tag="di")
        nc.vector.select(d_i[:], mask[:], c_neg[:], c_i[:])
        d_f = pool.tile([128, 1], f32, tag="df")
        nc.vector.tensor_copy(d_f[:], d_i[:])

        negv0 = pool.tile([128, 1], f32, tag="negv0")
        nc.vector.tensor_scalar_mul(negv0[:], vals[:, 0:1], -1.0)
        expv = pool.tile([128, 24], f32, tag="expv")
        nc.scalar.activation(expv[:, 0:22], vals[:, 0:22], AF.Exp, bias=negv0[:])
        den = pool.tile([128, 1], f32, tag="den")
        nc.vector.tensor_reduce(den[:], expv[:, 0:22], AX.X, AL.add)
        rden = pool.tile([128, 1], f32, tag="rden")
        nc.vector.reciprocal(rden[:], den[:])
        wgt = pool.tile([128, 1], f32, tag="wgt")
        nc.vector.tensor_mul(wgt[:], expv[:, 21:22], rden[:])

        # ---------------- V spectra h2/h3 ----------------
        vqk_b = pool2.tile([128, H * NF], bf16, tag="qkt")
        vhalf[1] = vqk_b
        v_proj([2, 3], vqk_b)

        # ---------------- phases ----------------
        negpi = pool.tile([128, 1], f32, tag="negpi")
        nc.gpsimd.memset(negpi[:], float(-np.pi))
        io_i = pool.tile([128, NF], i32, tag="scr_a")
        nc.gpsimd.iota(io_i[:], pattern=[[1, NF]], base=0, channel_multiplier=0)
        io_f = pool.tile([128, NF], f32, tag="scr_b")
        nc.vector.tensor_copy(io_f[:], io_i[:])
        cw = pool.tile([128, NF], bf16, tag="cw")
        sw = pool.tile([128, NF], bf16, tag="sw")
        # -Sin(pi/1024 * ((f*d + off)&2047) - pi) = sin/cos(2*pi*f*d/2048); the
        # leading minus is folded into the -2/T weight scale below.
        for off, dst in ((0.0, sw), (512.0, cw)):
            mf = pool2.tile([128, NF], f32, tag="ptmp")
            if off == 0.0:
                nc.vector.tensor_scalar(mf[:], io_f[:], d_f[:], None, AL.mult)
            else:
                nc.vector.tensor_scalar(mf[:], io_f[:], d_f[:], off, AL.mult, AL.add)
            mi = pool2.tile([128, NF], i32, tag="ptmp")
            nc.vector.tensor_copy(mi[:], mf[:])
            nc.vector.tensor_scalar(mi[:], mi[:], 2047, None, AL.bitwise_and)
            mf2 = pool2.tile([128, NF], f32, tag="ptmp")
            nc.vector.tensor_copy(mf2[:], mi[:])
            ph = pool2.tile([128, NF], f32, tag="ptmp")
            nc.scalar.activation(ph[:], mf2[:], AF.Sin,
                                 scale=float(np.pi / 1024.0), bias=negpi[:])
            nc.vector.tensor_scalar(dst[:], ph[:], wgt[:], -2.0 / T, AL.mult, AL.mult)
        swn = pool.tile([128, NF], bf16, tag="swn")
        nc.vector.tensor_scalar_mul(swn[:], sw[:], -1.0)
        # nyquist scale: (1-2*(d&1)) * wgt / T
        par_i = pool.tile([128, 1], i32, tag="par")
        nc.vector.tensor_scalar(par_i[:], d_i[:], 1, None, AL.bitwise_and)
        parf = pool.tile([128, 1], f32, tag="parf")
        nc.vector.tensor_copy(parf[:], par_i[:])
        nc.vector.tensor_scalar(parf[:], parf[:], -2.0, 1.0, AL.mult, AL.add)
        nys = pool.tile([128, 1], f32, tag="nys")
        nc.vector.tensor_scalar(nys[:], parf[:], wgt[:], 1.0 / T, AL.mult, AL.mult)

        # ---------------- V spectra + phase multiply (all h) ----------------
        gre_t = {}
        gim_t = {}
        gn_cols = pool.tile([128, H], bf16, tag="gncols")
        for h in range(H):
            vt = vhalf[h // 2]
            vre = vt[:, (2 * (h % 2)) * NF:(2 * (h % 2) + 1) * NF]
            vim = vt[:, (2 * (h % 2) + 1) * NF:(2 * (h % 2) + 2) * NF]
            nc.vector.tensor_mul(gn_cols[:, h:h + 1], vn_cols[:, h:h + 1], nys[:])

            m1 = pool.tile([128, NF], bf16, tag=f"gre{h}")
            m2 = pool.tile([128, NF], bf16, tag=f"gren{h}")
            m3 = pool.tile([128, NF], bf16, tag=f"gim{h}")
            m4 = pool.tile([128, NF], bf16, tag=f"gimn{h}")
            nc.vector.tensor_mul(m1[:], vre, cw[:])
            nc.vector.tensor_mul(m2[:], vim, swn[:])
            nc.vector.tensor_mul(m3[:], vre, sw[:])
            nc.vector.tensor_mul(m4[:], vim, cw[:])
            gre_t[h] = (m1, m2)
            gim_t[h] = (m3, m4)

        # ---------------- output projection ----------------
        of_re = pool.tile([128, NCH * E], f32r, tag="ofre")
        of_im = pool.tile([128, NCH * E], f32r, tag="ofim")
        for j in range(NCH):
            for c in range(2):
                of_ps = ppool.tile([128, 512], f32, tag="bank")
                gt = gre_t if c == 0 else gim_t
                for h in range(H):
                    ga, gb = gt[h]
                    nc.tensor.matmul(of_ps[:, 0:128], ga[:, j * 128:(j + 1) * 128],
                                     wo_sb[:, h * E:(h + 1) * E],
                                     start=(h == 0), stop=False)
                    nc.tensor.matmul(of_ps[:, 0:128], gb[:, j * 128:(j + 1) * 128],
                                     wo_sb[:, h * E:(h + 1) * E],
                                     start=False, stop=(h == H - 1))
                dst = of_re if c == 0 else of_im
                nc.vector.tensor_copy(dst[:, j * E:(j + 1) * E], of_ps[:, 0:128])
        ofn_ps = prow.tile([1, E], f32, tag="row")
        for h in range(H):
            nc.tensor.matmul(ofn_ps[:], gn_cols[:, h:h + 1], wo_sb[:, h * E:(h + 1) * E],
                             start=(h == 0), stop=(h == H - 1))
        ofn_row = pool.tile([1, E], f32r, tag="ofnrow")
        nc.vector.tensor_copy(ofn_row[:], ofn_ps[:])

        # ---------------- final inverse ----------------
        for s in range(2):
            sl = slice(s * 512, (s + 1) * 512)
            ep_ps = ppool.tile([128, 512], f32, tag="bank")
            op_ps = ppool.tile([128, 512], f32, tag="bank")
            for j in range(NCH):
                nc.tensor.matmul(ep_ps[:], of_re[:, j * E:(j + 1) * E],
                                 cs_sb[:, j * NF + s * 512: j * NF + (s + 1) * 512],
                                 start=(j == 0), stop=False)
            nc.tensor.matmul(ep_ps[:], ofn_row[:], altf_sb[:, sl], start=False, stop=False)
            # -0.5 * Of_re[0, f''] constant-in-t correction (f=0 weight fix)
            nc.tensor.matmul(ep_ps[:], of_re[0:1, 0:E], mhrow_sb[:, sl], start=False, stop=True)
            for j in range(NCH):
                nc.tensor.matmul(op_ps[:], of_im[:, j * E:(j + 1) * E],
                                 sn_sb[:, j * NF + s * 512: j * NF + (s + 1) * 512],
                                 start=(j == 0), stop=(j == NCH - 1))
            ep_sb = pool2.tile([128, 512], f32, tag="ecp")
            nc.scalar.copy(ep_sb[:], ep_ps[:])
            out_lo = pool2.tile([128, 512], f32, tag="outlo")
            out_hi = pool2.tile([128, 512], f32, tag="outlo")
            nc.vector.scalar_tensor_tensor(out_lo[:], ep_sb[:], bo_sb[:], op_ps[:], AL.add, AL.subtract)
            nc.vector.scalar_tensor_tensor(out_hi[:], ep_sb[:], bo_sb[:], op_ps[:], AL.add, AL.add)
            nc.sync.dma_start(lo_d[:, sl], out_lo[:])
            nc.sync.dma_start(hi_d[:, sl], out_hi[:])
        # t = 1024 row
        o1_ps = prow.tile([128, 1], f32, tag="row")
        for j in range(NCH):
            nc.tensor.matmul(o1_ps[:], c32(of_re[:, j * E:(j + 1) * E]), c32(altp_sb[:]),
                             start=(j == 0), stop=False)
        nc.tensor.matmul(o1_ps[:], c32(ofn_row[:]), c32(one_sb[:]), start=False, stop=False)
        nc.tensor.matmul(o1_ps[:], c32(of_re[0:1, 0:E]), c32(mhalf_sb[:]), start=False, stop=True)
        o1_sb = pool.tile([128, 1], f32, tag="o1sb")
        nc.vector.tensor_scalar(o1_sb[:], o1_ps[:], bo_sb[:], None, AL.add)
        nc.sync.dma_start(o1024_d[:], o1_sb[:])

    nc.compile()
    return nc


def _get_nc():
    if "nc" not in _CACHE:
        _wire_ntff_hook()
        _CACHE["nc"] = _build()
    return _CACHE["nc"]


def kernel(hidden_states, wq, bq, wk, bk, wv, bv, wo, bo):
    global LAST_EXEC_NS
    nc = _get_nc()
    consts = _CACHE.setdefault("consts", _host_consts())

    hs = np.ascontiguousarray(hidden_states, dtype=np.float32)
    wqk = np.concatenate([wq.transpose(2, 0, 1), wk.transpose(2, 0, 1)], axis=2)  # [H, E, 256]
    wv_h = np.ascontiguousarray(wv.transpose(2, 0, 1), dtype=np.float32)          # [H, E, E]
    wo_h = np.ascontiguousarray(wo.transpose(1, 0, 2), dtype=np.float32)          # [H, E, E]
    bqk = np.concatenate([(T * bq.T).reshape(-1), (T * bk.T).reshape(-1)])[None, :].astype(np.float32)  # [1, 2*H*E]
    bv_s = np.ascontiguousarray(T * bv, dtype=np.float32)                         # [E, H]
    bo_c = np.ascontiguousarray(bo, dtype=np.float32)[:, None]                    # [E, 1]

    in_maps = []
    for b in range(B):
        x = hs[b]
        xr = np.concatenate([x[0:1], x[:0:-1]])[:NF]
        in_maps.append({
            "x": x, "xr": np.ascontiguousarray(xr),
            "cs": consts["cs"], "sn": consts["sn"], "altf": consts["altf"],
            "altp": consts["altp"], "one": consts["one"], "mhalf": consts["mhalf"],
            "mhrow": consts["mhrow"],
            "wqk": np.ascontiguousarray(wqk, dtype=np.float32), "wv": wv_h,
            "wo": wo_h, "bqk": bqk, "bv": bv_s, "bo": bo_c,
        })

    trace = bool(int(os.environ.get("BASS_KERNEL_TRACE", "0")))
    res = run_bass_kernel_spmd(nc, in_maps, core_ids=list(range(B)), trace=trace)
    LAST_EXEC_NS = res.exec_time_ns
    _CACHE["last_res"] = res

    out = np.empty((B, T, E), dtype=np.float32)
    for b in range(B):
        r = res.results[b]
        out[b, 0:NF] = r["out_lo"].T
        out[b, NF] = r["out_1024"][:, 0]
        out[b, NF + 1:] = r["out_hi"][:, 1:NF][:, ::-1].T
    return out
